# revision 43
# baseline (speedup 1.0000x reference)
"""Trainium2 Bass kernel for nn_AffinityBiFC.

Reference computation (B=4, N=M=128, D=256, BD=1024):
    t  = einsum('bnd,dek->bnek', X, A)
    bi = einsum('bnek,bme->bnmk', t, Y)
    S  = einsum('bnmk,ok->bnmo', bi, W) + b        -> S[..., 0]  [B, N, M]

Algebraic collapse (exact reassociation):
    Aw[d, e] = sum_k A[d, e, k] * W[0, k]          # one streaming pass over A
    S[b]     = X[b] @ Aw @ Y[b].T + b              # tiny matmuls

Sharding: A is split over its first (d) axis across the 8 cores.  Each core
streams its 32 d-rows (16.75 MB as fp16), reduces them to Aw_c[32, 256],
computes its partial S_c = (X[:, :, rows_c] @ Aw_c) @ Y^T locally, and writes
S_c out.  The host sums the 8 partials and adds the bias — no device
collectives at all (the old AllGather-based design spent ~25us on the final
collective plus a ~48us serial tail).

Per-core pipeline (the 16.75 MB fp16 A stream feeds the PE directly):
  - Host packs A_c as [kp=128, dl=32, kc=8, e=256] fp16 (k = kc*128 + kp), so
    k lives on SBUF partitions and each DMA group is 128 fully-contiguous
    per-partition runs.
  - The k-reduction is a single PE op per (chunk, kc): stationary = the W
    column for that kc, replicated to all 128 PE columns, moving = the raw
    A block, so every psum row accumulates sum_kp W[kc*128+kp] * A[kp, de]
    over the 8 kc-blocks (row 0 is consumed).  No DVE pass at all: the
    per-matmul LDWEIGHTS cannot be elided on this toolchain
    (enable-ldw-opt=false), so reloading W as the stationary is free, and
    the A*W products happen in fp32 MACs.  Note: the PE clock sits at
    ~1.4 GHz whenever the HBM stream is active (shared power envelope,
    confirmed by the clock ramping to 2.4 GHz the moment the stream ends
    in every trace) — the PE/stream balance here is tuned for that.
  - ACT stages psum row 0 into aw_flat; rows 0..28 are rebuilt into the
    d-partitioned layout through a DRAM bounce mid-stream (an SBUF->SBUF
    partition scatter miscompiles on HW; engines cannot write at a
    partition offset).  The last 4 rows never bounce at all: T is closed
    by rank-1 matmuls (contraction dim = 1) reading aw_flat and a
    partition-0-staged X slice directly, so the tail has zero DMA
    dependencies and runs in the post-stream fast-clock drain.
  - Final: T = Aw_c^T X_c^T on PE, then S_c[b] = T^T Y_b^T, one fp16 copy,
    one 128 KB output DMA.
  - The W*32 / X/32 staging (exact power-of-two rescale, S unchanged) is
    kept from the earlier fp16-scratch design; with fp32 MACs it is no
    longer load-bearing but remains harmless.
"""

import numpy as np

B, N, D, KD = 4, 128, 256, 1024
P = 128
C = 8                    # cores
DL = D // C              # 32 d-rows per core
KC = KD // P             # 8 k-blocks
GROUPS = [1, 1, 2, 4, 4, 4, 4, 4, 4, 2, 1, 1]    # d-rows per DMA (ramp both ends)
assert sum(GROUPS) == DL
XSCALE = 32.0            # host stages W*32 and X/32 to keep A*W out of fp16 subnormals

_cached = {}


def _build_program():
    import concourse.bass as bass
    import concourse.mybir as mybir
    import concourse.tile as tile
    from concourse import bacc

    fp32 = mybir.dt.float32
    fp16 = mybir.dt.float16

    nc = bacc.Bacc(
        "TRN2",
        target_bir_lowering=False,
        debug=False,
        num_devices=C,
    )

    # host-packed A shard: [kp, dl, kc, e] fp16, k = kc*128 + kp
    a_sh = nc.dram_tensor("a_sh", [P, DL, KC, D], fp16, kind="ExternalInput").ap()
    w_in = nc.dram_tensor("w_in", [P, KC], fp16, kind="ExternalInput").ap()   # W[kc*128+kp]*32
    # W columns replicated to all 128 PE columns: full-array activity keeps
    # the HAM clock governor from seeing the PE as idle (the single-column
    # stationary left 127/128 of the array dark, likely pinning the clock
    # at the throttled 1.4 GHz state)
    wrp_in = nc.dram_tensor("wrp_in", [P, KC, P], fp16, kind="ExternalInput").ap()
    xt_in = nc.dram_tensor("xt_in", [DL, B, N], fp16, kind="ExternalInput").ap()  # (X/32)^T local rows
    xdf_in = nc.dram_tensor("xdf_in", [1, 4 * B * N], fp16, kind="ExternalInput").ap()  # rows 28..32, partition 0
    yt_in = nc.dram_tensor("yt_in", [D, B, N], fp16, kind="ExternalInput").ap()   # Y^T [e, b, m]
    out = nc.dram_tensor("out", [B, N, N], fp16, kind="ExternalOutput").ap()
    DEBUG = _cached.get("debug", False)
    if DEBUG:
        dbg_ones = nc.dram_tensor("dbg_ones", [P, P], fp16, kind="ExternalOutput").ap()
        dbg_scr0 = nc.dram_tensor("dbg_scr0", [P, KC, D], fp16, kind="ExternalOutput").ap()
        dbg_awflat = nc.dram_tensor("dbg_awflat", [1, DL * D], fp16, kind="ExternalOutput").ap()
        dbg_aw = nc.dram_tensor("dbg_aw", [DL, D], fp16, kind="ExternalOutput").ap()
        dbg_tT = nc.dram_tensor("dbg_tT", [P, 2, B, N], fp16, kind="ExternalOutput").ap()

    with tile.TileContext(nc) as tc:
        with (
            tc.tile_pool(name="apool", bufs=5) as apool,
            tc.tile_pool(name="sbuf", bufs=1) as sbuf,
            tc.tile_pool(name="pred", bufs=3, space="PSUM") as pred,
            tc.tile_pool(name="pfin", bufs=1, space="PSUM") as pfin,
            tc.tile_pool(name="dram", bufs=1, space="DRAM") as dram,
        ):
            # small inputs on the gpsimd SWDGE ring; sync ring stays on the
            # A stream.  The replicated-W stationary loads FIRST: the very
            # first reduce matmul depends on it, and PE start time is on the
            # critical path 1:1.
            wrp_sb = sbuf.tile([P, KC, P], fp16)
            nc.gpsimd.dma_start(wrp_sb[:, 0:1], wrp_in[:, 0:1])   # kc=0 gates the first matmul
            nc.gpsimd.dma_start(wrp_sb[:, 1:], wrp_in[:, 1:])
            xt_sb = sbuf.tile([DL, B, N], fp16)
            nc.gpsimd.dma_start(xt_sb[:], xt_in[:])
            xdf_sb = sbuf.tile([1, 4 * B * N], fp16)
            nc.gpsimd.dma_start(xdf_sb[:], xdf_in[:])
            yt_sb = sbuf.tile([P, 2, B, N], fp16)   # [e_lo, ec, b, m]
            nc.gpsimd.dma_start(yt_sb[:], yt_in.rearrange("(ec p) b m -> p ec b m", p=P))

            ones = sbuf.tile([P, P], fp16)
            nc.gpsimd.memset(ones[:], 1.0)

            aw_flat = sbuf.tile([1, DL * D], fp16)   # Aw staging on partition 0, (dl, e) order
            aw_sb = sbuf.tile([28, D], fp16)
            aw_dram = dram.tile([1, 28 * D], fp16)

            r0 = 0
            for g, r in enumerate(GROUPS):
                at = apool.tile([P, 4, KC, D], fp16, tag="a", name=f"at{g}")
                if g == 0:
                    # split the first group's load by kc halves: the kc 0..3
                    # matmuls can issue ~0.7us sooner, and the PE start time
                    # is on the critical path 1:1 (PE trails the stream)
                    nc.sync.dma_start(at[:, :r, : KC // 2], a_sh[:, r0 : r0 + r, : KC // 2])
                    nc.sync.dma_start(at[:, :r, KC // 2 :], a_sh[:, r0 : r0 + r, KC // 2 :])
                else:
                    nc.sync.dma_start(at[:, :r], a_sh[:, r0 : r0 + r])
                # PE scale+reduce in one op: stationary = the W column for
                # this kc ([128, 1] fp16), moving = the raw A block, so
                # psum[0, de] += sum_kp W[kc*128+kp] * A[kp, de].  The DVE
                # scaling pass is gone entirely (the per-matmul LDWEIGHTS was
                # unavoidable anyway, so the W-stationary reload is free), and
                # the A*W products now happen in fp32 MACs instead of rounding
                # through an fp16 scratch.
                for c0 in range(0, r, 2):
                    cw = min(2, r - c0)
                    ps = pred.tile([P, 2 * D], fp32, tag="ps", name=f"ps{g}_{c0}")
                    for kc in range(KC):
                        nc.tensor.matmul(
                            ps[:, : cw * D],
                            lhsT=wrp_sb[:, kc, :],
                            rhs=at[:, c0 : c0 + cw, kc],
                            start=(kc == 0),
                            stop=(kc == KC - 1),
                        )
                    # all psum rows equal -> ACT stages row 0 (fp32->fp16 cast)
                    row = r0 + c0
                    nc.scalar.activation(
                        out=aw_flat[0:1, row * D : (row + cw) * D],
                        in_=ps[0:1, : cw * D],
                        func=mybir.ActivationFunctionType.Copy,
                    )
                r0 += r
                if r0 == 28:
                    # piecewise Aw rebuild: rows 0..28 bounce through DRAM
                    # mid-stream so the tail only carries the last 4 rows
                    nc.gpsimd.dma_start(
                        aw_dram[0:1, : 28 * D], aw_flat[0:1, : 28 * D]
                    )
                    nc.gpsimd.dma_start(
                        aw_sb[:28, :],
                        aw_dram[0:1, : 28 * D].rearrange("o (r e) -> (o r) e", r=28),
                    )

            # close T: rows 0..28 via the rebuilt aw_sb, rows 28..32 via
            # rank-1 matmuls reading aw_flat directly on partition 0
            # (contraction dim = 1) — no tail DMA roundtrip at all, and the
            # rank-1 closes run in the post-stream fast-clock drain.
            psT = [pfin.tile([P, B * N], fp32, name=f"psT{ec}") for ec in range(2)]
            for ec in range(2):
                nc.tensor.matmul(
                    psT[ec],
                    lhsT=aw_sb[:, ec * P : (ec + 1) * P],
                    rhs=xt_sb[:28],
                    start=True,
                    stop=False,
                )
            for row in range(28, DL):
                for ec in range(2):
                    nc.tensor.matmul(
                        psT[ec],
                        lhsT=aw_flat[0:1, row * D + ec * P : row * D + ec * P + P],
                        rhs=xdf_sb[0:1, (row - 28) * B * N : (row - 27) * B * N],
                        start=False,
                        stop=(row == DL - 1),
                    )
            tT = sbuf.tile([P, 2, B, N], fp16)   # [e_lo, ec, b, n]
            nc.scalar.activation(
                out=tT[:, 0], in_=psT[0][:, :],
                func=mybir.ActivationFunctionType.Copy,
            )
            nc.vector.tensor_copy(tT[:, 1], psT[1][:, :])  # DVE is idle; runs beside ACT
            psS = pfin.tile([P, B, N], fp32)     # [n, b, m]
            s_sb = sbuf.tile([P, B, N], fp16)
            for b in range(B):
                for ec in range(2):
                    nc.tensor.matmul(
                        psS[:, b, :],
                        lhsT=tT[:, ec, b, :],
                        rhs=yt_sb[:, ec, b, :],
                        start=(ec == 0),
                        stop=(ec == 1),
                    )
                # copy batch b while batch b+1's matmuls run, and ship it
                # immediately: only the LAST 32 KB write's completion receipt
                # sits on the critical path instead of a full 128 KB DMA
                nc.scalar.activation(
                    out=s_sb[:, b], in_=psS[:, b, :],
                    func=mybir.ActivationFunctionType.Copy,
                )
                nc.sync.dma_start(
                    out[b].rearrange("n m -> n m"), s_sb[:, b]
                )

            if DEBUG:
                nc.sync.dma_start(dbg_ones[:], ones[:])
                nc.sync.dma_start(dbg_awflat[:], aw_flat[:])
                nc.sync.dma_start(dbg_aw[:], aw_sb[:])
                nc.sync.dma_start(dbg_tT[:], tT[:])

    nc.compile()
    return nc


def _get_program():
    if "nc" not in _cached:
        _cached["nc"] = _build_program()
    return _cached["nc"]


def _run(X, Y, A, W, b, trace=False, **trace_kwargs):
    from concourse.bass_utils import run_bass_kernel_spmd

    nc = _get_program()

    A = np.asarray(A, dtype=np.float32)
    W = np.asarray(W, dtype=np.float32)
    X = np.asarray(X, dtype=np.float32)
    Y = np.asarray(Y, dtype=np.float32)

    # W * 32 laid out [kp, kc]; X / 32 transposed to [d, b, n] (exact 2^5 rescale)
    w_cols = np.ascontiguousarray(
        (W.reshape(KC, P) * np.float32(XSCALE)).T, dtype=np.float16
    )
    w_rep_pe = np.ascontiguousarray(
        np.broadcast_to(w_cols[:, :, None], (P, KC, P)), dtype=np.float16
    )
    xt = np.ascontiguousarray(
        (X / np.float32(XSCALE)).transpose(2, 0, 1), dtype=np.float16
    )  # [d, b, n]
    yt = np.ascontiguousarray(Y.transpose(2, 0, 1), dtype=np.float16)  # [e, b, m]

    in_maps = []
    for c in range(C):
        rows = slice(c * DL, (c + 1) * DL)
        # [dl, e, k] -> [kp, dl, kc, e]
        a_perm = np.ascontiguousarray(
            A[rows].reshape(DL, D, KC, P).transpose(3, 0, 2, 1), dtype=np.float16
        )
        in_maps.append(
            {
                "a_sh": a_perm,
                "w_in": w_cols,
                "wrp_in": w_rep_pe,
                "xt_in": np.ascontiguousarray(xt[rows]),
                "xdf_in": np.ascontiguousarray(
                    xt[c * DL + 28 : (c + 1) * DL].reshape(1, 4 * B * N)
                ),
                "yt_in": yt,
            }
        )

    res = run_bass_kernel_spmd(nc, in_maps, list(range(C)), trace=trace, **trace_kwargs)
    # per-core outputs are partial sums over d; host unshard = sum + bias
    out = np.zeros((B, N, N), dtype=np.float32)
    for c in range(C):
        out += np.asarray(res.results[c]["out"], dtype=np.float32)
    out += np.float32(np.asarray(b).reshape(-1)[0])
    return out, res


def kernel(X, Y, A, W, b):
    out, _ = _run(X, Y, A, W, b, trace=False)
    return out


# revision 44
# speedup vs baseline: 1.0343x; 1.0343x over previous
"""Trainium2 Bass kernel for nn_AffinityBiFC.

Reference computation (B=4, N=M=128, D=256, BD=1024):
    t  = einsum('bnd,dek->bnek', X, A)
    bi = einsum('bnek,bme->bnmk', t, Y)
    S  = einsum('bnmk,ok->bnmo', bi, W) + b        -> S[..., 0]  [B, N, M]

Algebraic collapse (exact reassociation):
    Aw[d, e] = sum_k A[d, e, k] * W[0, k]          # one streaming pass over A
    S[b]     = X[b] @ Aw @ Y[b].T + b              # tiny matmuls

Sharding: A is split over its first (d) axis across the 8 cores.  Each core
streams its 32 d-rows (16.75 MB as fp16), reduces them to Aw_c[32, 256],
computes its partial S_c = (X[:, :, rows_c] @ Aw_c) @ Y^T locally, and writes
S_c out.  The host sums the 8 partials and adds the bias — no device
collectives at all (the old AllGather-based design spent ~25us on the final
collective plus a ~48us serial tail).

Per-core pipeline (the 16.75 MB fp16 A stream feeds the PE directly):
  - Host packs A_c as [kp=128, dl=32, kc=8, e=256] fp16 (k = kc*128 + kp), so
    k lives on SBUF partitions and each DMA group is 128 fully-contiguous
    per-partition runs.
  - The k-reduction is a single PE op per (chunk, kc): stationary = the W
    column for that kc, replicated to all 128 PE columns, moving = the raw
    A block, so every psum row accumulates sum_kp W[kc*128+kp] * A[kp, de]
    over the 8 kc-blocks (row 0 is consumed).  No DVE pass at all: the
    per-matmul LDWEIGHTS cannot be elided on this toolchain
    (enable-ldw-opt=false), so reloading W as the stationary is free, and
    the A*W products happen in fp32 MACs.  Note: the PE clock sits at
    ~1.4 GHz whenever the HBM stream is active (shared power envelope,
    confirmed by the clock ramping to 2.4 GHz the moment the stream ends
    in every trace) — the PE/stream balance here is tuned for that.
  - ACT stages psum row 0 into aw_flat; rows 0..28 are rebuilt into the
    d-partitioned layout through a DRAM bounce mid-stream (an SBUF->SBUF
    partition scatter miscompiles on HW; engines cannot write at a
    partition offset).  The last 4 rows never bounce at all: T is closed
    by rank-1 matmuls (contraction dim = 1) reading aw_flat and a
    partition-0-staged X slice directly, so the tail has zero DMA
    dependencies and runs in the post-stream fast-clock drain.
  - Final: T = Aw_c^T X_c^T on PE, then S_c[b] = T^T Y_b^T, one fp16 copy,
    one 128 KB output DMA.
  - The W*32 / X/32 staging (exact power-of-two rescale, S unchanged) is
    kept from the earlier fp16-scratch design; with fp32 MACs it is no
    longer load-bearing but remains harmless.
"""

import numpy as np

B, N, D, KD = 4, 128, 256, 1024
P = 128
C = 8                    # cores
DL = D // C              # 32 d-rows per core
KC = KD // P             # 8 k-blocks
GROUPS = [1, 1, 2, 4, 4, 4, 4, 4, 4, 2, 1, 1]    # d-rows per DMA (ramp both ends)
assert sum(GROUPS) == DL
XSCALE = 32.0            # host stages W*32 and X/32 to keep A*W out of fp16 subnormals

_cached = {}


def _build_program():
    import concourse.bass as bass
    import concourse.mybir as mybir
    import concourse.tile as tile
    from concourse import bacc

    fp32 = mybir.dt.float32
    fp16 = mybir.dt.float16

    nc = bacc.Bacc(
        "TRN2",
        target_bir_lowering=False,
        debug=False,
        num_devices=C,
    )

    # host-packed A shard: [kp, dl, kc, e] fp16, k = kc*128 + kp
    a_sh = nc.dram_tensor("a_sh", [P, DL, KC, D], fp16, kind="ExternalInput").ap()
    # W columns replicated to all 128 PE columns: full-array activity keeps
    # the HAM clock governor from seeing the PE as idle (the single-column
    # stationary left 127/128 of the array dark, likely pinning the clock
    # at the throttled 1.4 GHz state)
    wrp_in = nc.dram_tensor("wrp_in", [P, KC, P], fp16, kind="ExternalInput").ap()
    xt_in = nc.dram_tensor("xt_in", [DL, B, N], fp16, kind="ExternalInput").ap()  # (X/32)^T local rows
    xdf_in = nc.dram_tensor("xdf_in", [1, 4 * B * N], fp16, kind="ExternalInput").ap()  # rows 28..32, partition 0
    yt_in = nc.dram_tensor("yt_in", [D, B, N], fp16, kind="ExternalInput").ap()   # Y^T [e, b, m]
    out = nc.dram_tensor("out", [B, N, N], fp16, kind="ExternalOutput").ap()
    DEBUG = _cached.get("debug", False)
    if DEBUG:
        dbg_ones = nc.dram_tensor("dbg_ones", [P, P], fp16, kind="ExternalOutput").ap()
        dbg_scr0 = nc.dram_tensor("dbg_scr0", [P, KC, D], fp16, kind="ExternalOutput").ap()
        dbg_awflat = nc.dram_tensor("dbg_awflat", [1, DL * D], fp16, kind="ExternalOutput").ap()
        dbg_aw = nc.dram_tensor("dbg_aw", [DL, D], fp16, kind="ExternalOutput").ap()
        dbg_tT = nc.dram_tensor("dbg_tT", [P, 2, B, N], fp16, kind="ExternalOutput").ap()

    with tile.TileContext(nc) as tc:
        with (
            tc.tile_pool(name="apool", bufs=5) as apool,
            tc.tile_pool(name="sbuf", bufs=1) as sbuf,
            tc.tile_pool(name="pred", bufs=3, space="PSUM") as pred,
            tc.tile_pool(name="pfin", bufs=1, space="PSUM") as pfin,
            tc.tile_pool(name="dram", bufs=1, space="DRAM") as dram,
        ):
            # small inputs on the gpsimd SWDGE ring; sync ring stays on the
            # A stream.  The replicated-W stationary loads FIRST: the very
            # first reduce matmul depends on it, and PE start time is on the
            # critical path 1:1.
            wrp_sb = sbuf.tile([P, KC, P], fp16)
            nc.gpsimd.dma_start(wrp_sb[:, 0:1], wrp_in[:, 0:1])   # kc=0 gates the first matmul
            nc.gpsimd.dma_start(wrp_sb[:, 1:], wrp_in[:, 1:])
            xt_sb = sbuf.tile([DL, B, N], fp16)
            nc.gpsimd.dma_start(xt_sb[:], xt_in[:])
            xdf_sb = sbuf.tile([1, 4 * B * N], fp16)
            nc.gpsimd.dma_start(xdf_sb[:], xdf_in[:])
            yt_sb = sbuf.tile([P, 2, B, N], fp16)   # [e_lo, ec, b, m]
            nc.gpsimd.dma_start(yt_sb[:], yt_in.rearrange("(ec p) b m -> p ec b m", p=P))

            aw_flat = sbuf.tile([1, DL * D], fp16)   # Aw staging on partition 0, (dl, e) order
            aw_sb = sbuf.tile([28, D], fp16)
            aw_dram = dram.tile([1, 28 * D], fp16)

            r0 = 0
            for g, r in enumerate(GROUPS):
                at = apool.tile([P, 4, KC, D], fp16, tag="a", name=f"at{g}")
                if g == 0:
                    # split the first group's load by kc halves: the kc 0..3
                    # matmuls can issue ~0.7us sooner, and the PE start time
                    # is on the critical path 1:1 (PE trails the stream)
                    nc.sync.dma_start(at[:, :r, : KC // 2], a_sh[:, r0 : r0 + r, : KC // 2])
                    nc.sync.dma_start(at[:, :r, KC // 2 :], a_sh[:, r0 : r0 + r, KC // 2 :])
                else:
                    nc.sync.dma_start(at[:, :r], a_sh[:, r0 : r0 + r])
                # PE scale+reduce in one op: stationary = the W column for
                # this kc ([128, 1] fp16), moving = the raw A block, so
                # psum[0, de] += sum_kp W[kc*128+kp] * A[kp, de].  The DVE
                # scaling pass is gone entirely (the per-matmul LDWEIGHTS was
                # unavoidable anyway, so the W-stationary reload is free), and
                # the A*W products now happen in fp32 MACs instead of rounding
                # through an fp16 scratch.
                for c0 in range(0, r, 2):
                    cw = min(2, r - c0)
                    ps = pred.tile([P, 2 * D], fp32, tag="ps", name=f"ps{g}_{c0}")
                    for kc in range(KC):
                        nc.tensor.matmul(
                            ps[:, : cw * D],
                            lhsT=wrp_sb[:, kc, :],
                            rhs=at[:, c0 : c0 + cw, kc],
                            start=(kc == 0),
                            stop=(kc == KC - 1),
                        )
                    # all psum rows equal -> ACT stages row 0 (fp32->fp16 cast)
                    row = r0 + c0
                    nc.scalar.activation(
                        out=aw_flat[0:1, row * D : (row + cw) * D],
                        in_=ps[0:1, : cw * D],
                        func=mybir.ActivationFunctionType.Copy,
                    )
                r0 += r
                if r0 == 28:
                    # piecewise Aw rebuild: rows 0..28 bounce through DRAM
                    # mid-stream so the tail only carries the last 4 rows
                    nc.gpsimd.dma_start(
                        aw_dram[0:1, : 28 * D], aw_flat[0:1, : 28 * D]
                    )
                    nc.gpsimd.dma_start(
                        aw_sb[:28, :],
                        aw_dram[0:1, : 28 * D].rearrange("o (r e) -> (o r) e", r=28),
                    )

            # close T: rows 0..28 via the rebuilt aw_sb, rows 28..32 via
            # rank-1 matmuls reading aw_flat directly on partition 0
            # (contraction dim = 1) — no tail DMA roundtrip at all, and the
            # rank-1 closes run in the post-stream fast-clock drain.
            psT = [pfin.tile([P, B * N], fp32, name=f"psT{ec}") for ec in range(2)]
            for ec in range(2):
                nc.tensor.matmul(
                    psT[ec],
                    lhsT=aw_sb[:, ec * P : (ec + 1) * P],
                    rhs=xt_sb[:28],
                    start=True,
                    stop=False,
                )
            for row in range(28, DL):
                for ec in range(2):
                    nc.tensor.matmul(
                        psT[ec],
                        lhsT=aw_flat[0:1, row * D + ec * P : row * D + ec * P + P],
                        rhs=xdf_sb[0:1, (row - 28) * B * N : (row - 27) * B * N],
                        start=False,
                        stop=(row == DL - 1),
                    )
            tT = sbuf.tile([P, 2, B, N], fp16)   # [e_lo, ec, b, n]
            nc.scalar.activation(
                out=tT[:, 0], in_=psT[0][:, :],
                func=mybir.ActivationFunctionType.Copy,
            )
            nc.vector.tensor_copy(tT[:, 1], psT[1][:, :])  # DVE is idle; runs beside ACT
            psS = pfin.tile([P, B, N], fp32)     # [n, b, m]
            s_sb = sbuf.tile([P, B, N], fp16)
            for b in range(B):
                for ec in range(2):
                    nc.tensor.matmul(
                        psS[:, b, :],
                        lhsT=tT[:, ec, b, :],
                        rhs=yt_sb[:, ec, b, :],
                        start=(ec == 0),
                        stop=(ec == 1),
                    )
                # copy batch b while batch b+1's matmuls run, and ship it
                # immediately: only the LAST 32 KB write's completion receipt
                # sits on the critical path instead of a full 128 KB DMA
                nc.scalar.activation(
                    out=s_sb[:, b], in_=psS[:, b, :],
                    func=mybir.ActivationFunctionType.Copy,
                )
                nc.sync.dma_start(
                    out[b].rearrange("n m -> n m"), s_sb[:, b]
                )

            if DEBUG:
                nc.sync.dma_start(dbg_ones[:], ones[:])
                nc.sync.dma_start(dbg_awflat[:], aw_flat[:])
                nc.sync.dma_start(dbg_aw[:], aw_sb[:])
                nc.sync.dma_start(dbg_tT[:], tT[:])

    nc.compile()
    return nc


def _get_program():
    if "nc" not in _cached:
        _cached["nc"] = _build_program()
    return _cached["nc"]


def _run(X, Y, A, W, b, trace=False, **trace_kwargs):
    from concourse.bass_utils import run_bass_kernel_spmd

    nc = _get_program()

    A = np.asarray(A, dtype=np.float32)
    W = np.asarray(W, dtype=np.float32)
    X = np.asarray(X, dtype=np.float32)
    Y = np.asarray(Y, dtype=np.float32)

    # W * 32 laid out [kp, kc]; X / 32 transposed to [d, b, n] (exact 2^5 rescale)
    w_cols = np.ascontiguousarray(
        (W.reshape(KC, P) * np.float32(XSCALE)).T, dtype=np.float16
    )
    w_rep_pe = np.ascontiguousarray(
        np.broadcast_to(w_cols[:, :, None], (P, KC, P)), dtype=np.float16
    )
    xt = np.ascontiguousarray(
        (X / np.float32(XSCALE)).transpose(2, 0, 1), dtype=np.float16
    )  # [d, b, n]
    yt = np.ascontiguousarray(Y.transpose(2, 0, 1), dtype=np.float16)  # [e, b, m]

    in_maps = []
    for c in range(C):
        rows = slice(c * DL, (c + 1) * DL)
        # [dl, e, k] -> [kp, dl, kc, e]
        a_perm = np.ascontiguousarray(
            A[rows].reshape(DL, D, KC, P).transpose(3, 0, 2, 1), dtype=np.float16
        )
        in_maps.append(
            {
                "a_sh": a_perm,
                "wrp_in": w_rep_pe,
                "xt_in": np.ascontiguousarray(xt[rows]),
                "xdf_in": np.ascontiguousarray(
                    xt[c * DL + 28 : (c + 1) * DL].reshape(1, 4 * B * N)
                ),
                "yt_in": yt,
            }
        )

    res = run_bass_kernel_spmd(nc, in_maps, list(range(C)), trace=trace, **trace_kwargs)
    # per-core outputs are partial sums over d; host unshard = sum + bias
    out = np.zeros((B, N, N), dtype=np.float32)
    for c in range(C):
        out += np.asarray(res.results[c]["out"], dtype=np.float32)
    out += np.float32(np.asarray(b).reshape(-1)[0])
    return out, res


def kernel(X, Y, A, W, b):
    out, _ = _run(X, Y, A, W, b, trace=False)
    return out
